# revision 43
# baseline (speedup 1.0000x reference)
"""Multi-head attention (B=8, N=1024, C=768, H=12) on 8 TRN2 NeuronCores.

Data-parallel: one batch element per core. Feature-major on chip (no
transposes):

  qkT  = [w_q * scale ; w_k] @ x^T          -> [1536, 1024]  (feature-major)
  V    = x @ w_v^T                          -> [1024, 768]   (token-major)
  S^T  = K_h @ Q_h^T                        -> [128m, 512n] per (pair, m-tile)
  P^T  = exp(S^T)            (no max-sub: scores ~ N(0,1), fp32-safe)
  [out_h ; Z] = [V_h | 1]^T @ P^T           -> [65, 512]  (Z = softmax denom)
  attnT[h] = out_h * recip(Z bcast via K=2 matmul, both heads at once)
  y^T  = w_proj @ attnT + b                 -> [768, 1024]

All inputs are pre-cast to bf16 on the host (fp32 PSUM accumulation on
chip), so DMAs feed SBUF directly with no on-chip cast. The emission
order interleaves score matmuls (which feed ScalarE's exp stream) with
QKV/V/AV/proj "filler" matmuls so the PE never idles: TRN2's tensor
engine downclocks after any gap and takes ~3us to re-ramp, so density
is worth more than locality. A dummy-matmul warmup burst runs during
the initial DMA wait to pre-ramp the PE clock.
"""

import sys

if "/opt/trn_rl_repo" not in sys.path:
    sys.path.insert(0, "/opt/trn_rl_repo")

import numpy as np

import concourse.bass as bass  # noqa: F401
import concourse.mybir as mybir
import concourse.tile as tile
from concourse import bacc
from concourse.bass_utils import run_bass_kernel_spmd

F32 = mybir.dt.float32
F32R = mybir.dt.float32r
BF16 = mybir.dt.bfloat16
AF = mybir.ActivationFunctionType

B, N, C = 8, 1024, 768
H, D = 12, 64
SCALE = D ** -0.5
KT = C // 128            # 6 contraction tiles
NT = N // 128            # 8 token (m) tiles
NCH = N // 512           # 2 free-dim chunks of 512
NPAIR = H // 2           # 6 head pairs

_CACHE = {}


def build():
    nc = bacc.Bacc("TRN2", target_bir_lowering=False, debug=False, num_devices=8)

    xT_d = nc.dram_tensor("xT", [C, N], BF16, kind="ExternalInput")
    wqk_d = nc.dram_tensor("w_qkT", [12, 128, C], BF16, kind="ExternalInput")
    wv_d = nc.dram_tensor("w_vT", [C, C], BF16, kind="ExternalInput")
    wp_d = nc.dram_tensor("w_pT", [C, C], BF16, kind="ExternalInput")
    b_d = nc.dram_tensor("b_p", [C, 1], F32, kind="ExternalInput")
    out_d = nc.dram_tensor("out", [C, N], F32, kind="ExternalOutput")

    with tile.TileContext(nc) as tc:
        _body(nc, tc, xT_d, wqk_d, wv_d, wp_d, b_d, out_d)
    nc.compile()
    return nc


def _body(nc, tc, xT_d, wqk_d, wv_d, wp_d, b_d, out_d):
    from collections import deque
    from contextlib import ExitStack

    with ExitStack() as ctx:
        ctx.enter_context(
            nc.allow_low_precision(reason="bf16 matmul operands; accum stays f32")
        )
        const = ctx.enter_context(tc.tile_pool(name="const", bufs=1))
        x_pool = ctx.enter_context(tc.tile_pool(name="x", bufs=1))
        w_pool = ctx.enter_context(tc.tile_pool(name="w", bufs=1))
        qk_pool = ctx.enter_context(tc.tile_pool(name="qk", bufs=1))
        v_pool = ctx.enter_context(tc.tile_pool(name="v", bufs=1))
        attn_pool = ctx.enter_context(tc.tile_pool(name="attn", bufs=1))
        pt_pool = ctx.enter_context(tc.tile_pool(name="pt", bufs=36))
        zs_pool = ctx.enter_context(tc.tile_pool(name="zs", bufs=2))
        y_pool = ctx.enter_context(tc.tile_pool(name="y", bufs=3))
        ps_pool = ctx.enter_context(tc.tile_pool(name="ps", bufs=2, space="PSUM"))
        pss_pool = ctx.enter_context(tc.tile_pool(name="pss", bufs=2, space="PSUM"))
        pav_pool = ctx.enter_context(tc.tile_pool(name="pav", bufs=2, space="PSUM"))

        # ---------------- constants ----------------
        warm = const.tile([128, 512], BF16)
        nc.vector.memset(warm, 0.25)
        b_sb = const.tile([128, KT], F32)
        # ---------------- persistent SBUF tensors ----------------
        xT = [x_pool.tile([128, N], BF16, tag=f"x{i}", name=f"x{i}") for i in range(KT)]
        wqk = [w_pool.tile([128, KT * 128], BF16, tag=f"wqk{i}", name=f"wqk{i}") for i in range(12)]
        wv = [w_pool.tile([128, C], BF16, tag=f"wv{i}", name=f"wv{i}") for i in range(KT)]
        wp = [w_pool.tile([128, C], BF16, tag=f"wp{i}", name=f"wp{i}") for i in range(KT)]
        qkT = [qk_pool.tile([128, N], BF16, tag=f"qkT{i}", name=f"qkT{i}") for i in range(12)]
        v_sb = [v_pool.tile([128, H, 65], BF16, tag=f"v{i}", name=f"v{i}") for i in range(NT)]
        attnT = [attn_pool.tile([128, N], BF16, tag=f"at{i}", name=f"at{i}") for i in range(KT)]

        # ---------------- input DMAs (2 queues, consumption order) ----------------
        # sync queue: xT (needed first), then wv, b; scalar queue: wqk blocks
        # in the order QKV tiles consume them, then wp (needed last).
        for kt in range(KT):
            ksl = slice(kt * 128, (kt + 1) * 128)
            nc.sync.dma_start(out=xT[kt], in_=xT_d.ap()[ksl, :])
        for ot in (0, 6, 1, 7, 2, 8, 3, 9, 4, 10, 5, 11):
            nc.scalar.dma_start(
                out=wqk[ot].rearrange("p (k m) -> p k m", m=128),
                in_=wqk_d.ap()[ot],
            )
        for kt in range(KT):
            ksl = slice(kt * 128, (kt + 1) * 128)
            nc.sync.dma_start(out=wv[kt], in_=wv_d.ap()[ksl, :])
        for ot in range(KT):
            nc.sync.dma_start(
                out=b_sb[:, ot : ot + 1], in_=b_d.ap()[ot * 128 : (ot + 1) * 128, :]
            )
        for kt in range(KT):
            ksl = slice(kt * 128, (kt + 1) * 128)
            nc.scalar.dma_start(out=wp[kt], in_=wp_d.ap()[ksl, :])
        # V ones-columns (row 64 of each head slot)
        for mt in range(NT):
            nc.gpsimd.memset(v_sb[mt][:, :, 64:65], 1.0)

        # ---------------- more constants (gpsimd, after DMA issues) ----------------
        # Z-broadcast stationary: row 0 -> out partitions 0..63, row 64 ->
        # out partitions 64..127 (partition bases must be quadrant-aligned).
        zst_raw = const.tile([128, 128], F32)
        nc.gpsimd.memset(zst_raw, 0.0)
        nc.gpsimd.memset(zst_raw[0:1, 0:64], 1.0)
        nc.gpsimd.memset(zst_raw[64:65, 64:128], 1.0)
        zst = const.tile([128, 128], F32R)
        nc.gpsimd.tensor_copy(zst, zst_raw)
        # Z staging tiles (rows 0 and 64 carry the two heads' denominators;
        # the rest is zeroed once and multiplied by zero weights anyway).
        z2_zero = const.tile([128, 512], F32)
        nc.gpsimd.memset(z2_zero, 0.0)
        z2_bufs = [const.tile([128, 512], F32R, name=f"z2_{i}") for i in range(2)]
        for zb_ in z2_bufs:
            nc.gpsimd.tensor_copy(zb_, z2_zero)

        # ---------------- PE warmup (overlaps DMA wait) ----------------
        for i in range(12):
            pw = pss_pool.tile([128, 1024], F32, tag="pss", name=f"warm{i}")
            nc.tensor.matmul(
                pw[:, 0:512], lhsT=warm[:, 0:128], rhs=warm, start=True, stop=True
            )

        # ---------------- work units ----------------
        def qku(ot, nch):
            """One QKV output tile chunk: 6 matmuls + copy to qkT."""
            nsl = slice(nch * 512, (nch + 1) * 512)
            ps = ps_pool.tile([128, 512], F32, tag="ps", name=f"psqk_{ot}_{nch}")
            for kt in range(KT):
                nc.tensor.matmul(
                    ps,
                    lhsT=wqk[ot][:, kt * 128 : (kt + 1) * 128],
                    rhs=xT[kt][:, nsl],
                    start=(kt == 0),
                    stop=(kt == KT - 1),
                )
            nc.vector.tensor_copy(qkT[ot][:, nsl], ps)

        def v_unit(mt):
            """V for one token tile: x-tile stationary, wv moving (512+256)."""
            msl = slice(mt * 128, (mt + 1) * 128)
            for o0, ow in ((0, 512), (512, 256)):
                nh = ow // 64
                ps = ps_pool.tile([128, 512], F32, tag="ps", name=f"psv_{mt}_{o0}")
                for kt in range(KT):
                    nc.tensor.matmul(
                        ps[:, :ow],
                        lhsT=xT[kt][:, msl],
                        rhs=wv[kt][:, o0 : o0 + ow],
                        start=(kt == 0),
                        stop=(kt == KT - 1),
                    )
                dst = v_sb[mt][:, o0 // 64 : o0 // 64 + nh, 0:64]
                vsrc = ps[:, :ow].rearrange("p (h e) -> p h e", e=64)
                nc.vector.tensor_copy(dst, vsrc)

        pt_tiles = {}

        def s_unit(p, nch, g):
            """Scores + exp for one (pair, n-chunk, m-tile-pair): 4 matmuls +
            2 exps of [128, 1024] (bigger exps amortize ScalarE overhead)."""
            q_t = qkT[p]
            k_t = qkT[6 + p]
            nsl = slice(nch * 512, (nch + 1) * 512)
            for e in range(2):
                esl = slice(e * 64, e * 64 + 64)
                pss = pss_pool.tile(
                    [128, 1024], F32, tag="pss", name=f"pss_{p}_{nch}_{g}_{e}"
                )
                for j in range(2):
                    mt = 2 * g + j
                    msl = slice(mt * 128, (mt + 1) * 128)
                    nc.tensor.matmul(
                        pss[:, j * 512 : (j + 1) * 512],
                        lhsT=k_t[esl, msl],
                        rhs=q_t[esl, nsl],
                        start=True,
                        stop=True,
                    )
                pt = pt_pool.tile(
                    [128, 1024], BF16, tag="pt", name=f"pt_{p}_{nch}_{g}_{e}"
                )
                nc.scalar.activation(pt, pss, AF.Exp)
                pt_tiles[(p, nch, g, e)] = pt

        pav_cur = {}

        def av_unit(p, nch, mt):
            """AV accumulation for one m-tile of a block (2 matmuls)."""
            if mt == 0:
                pav_cur[(p, nch)] = [
                    pav_pool.tile([65, 512], F32, tag="pav", name=f"pav_{p}_{nch}_{e}")
                    for e in range(2)
                ]
            pav = pav_cur[(p, nch)]
            g, j = divmod(mt, 2)
            jsl = slice(j * 512, (j + 1) * 512)
            for e in range(2):
                h = 2 * p + e
                pt = pt_tiles[(p, nch, g, e)]
                if j == 1 and e == 1:
                    del pt_tiles[(p, nch, g, 0)], pt_tiles[(p, nch, g, 1)]
                nc.tensor.matmul(
                    pav[e],
                    lhsT=v_sb[mt][:, h, :],
                    rhs=pt[:, jsl],
                    start=(mt == 0),
                    stop=(mt == NT - 1),
                )

        norm_count = [0]
        tail_mode = [False]
        z2_of = {}

        def norm_a(p, nch):
            """First half of softmax normalization: stage Z rows (no PE)."""
            pav = pav_cur[(p, nch)]
            z2 = z2_bufs[norm_count[0] % 2]
            norm_count[0] += 1
            z2_of[(p, nch)] = z2
            if tail_mode[0]:
                nc.scalar.copy(out=z2[0:1, :], in_=pav[0][64:65, :])
                nc.scalar.copy(out=z2[64:65, :], in_=pav[1][64:65, :])
            else:
                nc.vector.tensor_copy(z2[0:1, :], pav[0][64:65, :])
                nc.vector.tensor_copy(z2[64:65, :], pav[1][64:65, :])

        def norm_b(p, nch, use_pss=False):
            """Second half: Z broadcast matmul + reciprocal + muls."""
            nsl = slice(nch * 512, (nch + 1) * 512)
            pav = pav_cur.pop((p, nch))
            z2 = z2_of.pop((p, nch))
            if use_pss:
                zbt = pss_pool.tile([128, 1024], F32, tag="pss", name=f"zb_{p}_{nch}")
                zb = zbt[:, 0:512]
            else:
                zb = ps_pool.tile([128, 512], F32, tag="ps", name=f"zb_{p}_{nch}")
            nc.tensor.matmul(zb, lhsT=zst, rhs=z2, start=True, stop=True)
            zr = zs_pool.tile([128, 512], F32, tag="zr", name=f"zr_{p}_{nch}")
            nc.vector.reciprocal_approx_fast(out=zr, in_=zb)
            for e in range(2):
                dst = attnT[p][e * 64 : e * 64 + 64, nsl]
                nc.vector.tensor_mul(dst, zr[e * 64 : e * 64 + 64, :], pav[e][0:64, :])

        def proj_unit(ot, nch):
            """Output projection tile: 6 matmuls + bias add (vector) + DMA out."""
            osl = slice(ot * 128, (ot + 1) * 128)
            nsl = slice(nch * 512, (nch + 1) * 512)
            ps = ps_pool.tile([128, 512], F32, tag="ps", name=f"psy_{ot}_{nch}")
            for kt in range(KT):
                nc.tensor.matmul(
                    ps,
                    lhsT=wp[kt][:, osl],
                    rhs=attnT[kt][:, nsl],
                    start=(kt == 0),
                    stop=(kt == KT - 1),
                )
            y = y_pool.tile([128, 512], F32, tag="y", name=f"y_{ot}_{nch}")
            if tail_mode[0]:
                nc.scalar.activation(y, ps, AF.Identity, bias=b_sb[:, ot : ot + 1])
            else:
                nc.vector.tensor_scalar_add(y, ps, b_sb[:, ot : ot + 1])
            nc.sync.dma_start(out=out_d.ap()[osl, nsl], in_=y)

        # ---------------- schedule ----------------
        # Emission order is engine program order. A coarse clock model
        # (pe_t = estimated PE stream position, sc_t = estimated ScalarE exp
        # completion) rations filler so the PE reaches each scores unit just
        # as its PSUM slot is freed by the exp stream -- never stalling
        # (which would also drop the PE out of max p-state).
        MM512, MM256, EXPNS, MARGIN = 215.0, 110.0, 1180.0, -500.0
        _stats = {"starved": 0, "starve_ns": 0.0}
        clock = {"pe": 2500.0, "sc": 0.0}
        pss_free = [0.0] * 12  # warmup allocations, free immediately
        exp_end = {}
        for _ in range(12):
            clock["pe"] += MM512

        blocks = [(p, nch) for nch in range(NCH) for p in range(NPAIR)]
        qku_done = set()

        def emit_qku(ot, nch):
            qku(ot, nch)
            qku_done.add((ot, nch))
            clock["pe"] += KT * MM512

        qku_queue = deque(
            (ot, nch)
            for p in range(1, NPAIR)
            for ot in (p, 6 + p)
            for nch in range(NCH)
        )
        v_queue = deque(range(NT))
        av_queue = deque()
        proj_queue = deque()
        v_emitted = 0
        norms_done = [0] * NCH
        normb_queue = deque()  # (p, nch, pe_when_staged)

        def emit_av(p, nch, mt):
            av_unit(p, nch, mt)
            clock["pe"] += 2 * MM512
            if mt == NT - 1:
                norm_a(p, nch)
                normb_queue.append((p, nch, clock["pe"]))

        def emit_normb(p, nch, use_pss=False):
            norm_b(p, nch, use_pss)
            clock["pe"] += MM512
            norms_done[nch] += 1
            if nch == 0 and norms_done[0] == NPAIR:
                proj_queue.extend((ot, 0) for ot in range(KT))

        def emit_proj(ot, nch):
            proj_unit(ot, nch)
            clock["pe"] += KT * MM512

        dummy_n = [0]

        def emit_dummy():
            # throwaway matmul: keeps the PE at max p-state through a gap
            # that has no real work (a stall would cost ~3us of half-clock)
            ps = ps_pool.tile([128, 512], F32, tag="ps", name=f"dum{dummy_n[0]}")
            dummy_n[0] += 1
            nc.tensor.matmul(
                ps, lhsT=warm[:, 0:128], rhs=warm, start=True, stop=True
            )
            clock["pe"] += MM512

        def emit_v():
            nonlocal v_emitted
            v_unit(v_queue.popleft())
            v_emitted += 1
            clock["pe"] += KT * (MM512 + MM256)

        def emit_filler(upcoming_pair):
            if normb_queue and (
                clock["pe"] >= normb_queue[0][2] + 1600 or len(normb_queue) >= 2
            ):
                p, nch, _ = normb_queue.popleft()
                emit_normb(p, nch)
                return True
            if qku_queue and qku_queue[0][0] in (upcoming_pair, 6 + upcoming_pair):
                emit_qku(*qku_queue.popleft())
                return True
            if v_queue:
                emit_v()
                return True
            if av_queue and v_emitted == NT:
                p, nch, mt = av_queue[0]
                g = mt // 2
                if mt == 0 and normb_queue:
                    # new block needs both pav slots: flush pending norm-b
                    bp, bnch, _ = normb_queue.popleft()
                    emit_normb(bp, bnch)
                    return True
                if exp_end[(p, nch, g, 1)] <= clock["pe"]:
                    av_queue.popleft()
                    emit_av(p, nch, mt)
                    return True
            if proj_queue:
                emit_proj(*proj_queue.popleft())
                return True
            if qku_queue:
                emit_qku(*qku_queue.popleft())
                return True
            return False

        def s_half(p, nch, g, e):
            """Scores matmuls + exp for one head of a score group, with
            filler emitted until the needed pss slot is modeled free."""
            need = pss_free[len(pss_free) - 2] - MARGIN
            while clock["pe"] < need:
                if not emit_filler(p):
                    _stats["starved"] += 1
                    _stats["starve_ns"] += need - clock["pe"]
                    _stats.setdefault("log", []).append(
                        (p, nch, g, e, round(need - clock["pe"]),
                         len(qku_queue), len(v_queue), len(av_queue),
                         len(proj_queue), len(normb_queue),
                         av_queue[0] if av_queue else None,
                         round(clock["pe"]))
                    )
                    break
            q_t = qkT[p]
            k_t = qkT[6 + p]
            nsl = slice(nch * 512, (nch + 1) * 512)
            esl = slice(e * 64, e * 64 + 64)
            pss = pss_pool.tile(
                [128, 1024], F32, tag="pss", name=f"pss_{p}_{nch}_{g}_{e}"
            )
            for j in range(2):
                mt = 2 * g + j
                msl = slice(mt * 128, (mt + 1) * 128)
                nc.tensor.matmul(
                    pss[:, j * 512 : (j + 1) * 512],
                    lhsT=k_t[esl, msl],
                    rhs=q_t[esl, nsl],
                    start=True,
                    stop=True,
                )
            clock["pe"] += 2 * MM512
            pt = pt_pool.tile(
                [128, 1024], BF16, tag="pt", name=f"pt_{p}_{nch}_{g}_{e}"
            )
            nc.scalar.activation(pt, pss, AF.Exp)
            end = max(clock["sc"], clock["pe"]) + EXPNS
            clock["sc"] = end
            pss_free.append(end)
            exp_end[(p, nch, g, e)] = end
            pt_tiles[(p, nch, g, e)] = pt

        # pair-0 Q/K first so the exp stream starts as early as possible
        for ot in (0, 6):
            for nch in range(NCH):
                emit_qku(ot, nch)

        for bi, (p, nch) in enumerate(blocks):
            nxt = blocks[bi + 1][0] if bi + 1 < len(blocks) else NPAIR - 1
            for ot in (p, 6 + p):
                for c in range(NCH):
                    if (ot, c) not in qku_done:
                        qku_queue.remove((ot, c))
                        emit_qku(ot, c)
            for g in range(NT // 2):
                for e in range(2):
                    s_half(p, nch, g, e)
            if bi >= 2:
                bp, bnch = blocks[bi - 2]
                av_queue.extend((bp, bnch, mt) for mt in range(NT))

        # ---------------- tail ----------------
        for bp, bnch in blocks[-2:]:
            av_queue.extend((bp, bnch, mt) for mt in range(NT))
        # drain AV (except the final block); between a block's norm-a (z2
        # copies) and norm-b (zb matmul) give the PE proj work so it never
        # waits on the copy/recip chain
        last = blocks[-1]
        while av_queue:
            p, nch, mt = av_queue[0]
            if (p, nch) == last:
                break
            av_queue.popleft()
            emit_av(p, nch, mt)
            if mt == NT - 1:
                if proj_queue:
                    emit_proj(*proj_queue.popleft())
                if normb_queue:
                    bp, bnch, _ = normb_queue.popleft()
                    emit_normb(bp, bnch, use_pss=True)
                if proj_queue:
                    emit_proj(*proj_queue.popleft())
        while proj_queue:
            emit_proj(*proj_queue.popleft())
        # final block: AV, then proj(nch=1) split into head (kt 0..4, does
        # not need the final norm) and tail (kt=5 + bias + DMA out) so the
        # PE has work while the last norm's vector chain completes. Only now
        # is ScalarE's exp backlog drained enough to take the copies/bias.
        tail_mode[0] = True
        while len(normb_queue) > 0 and av_queue:
            bp, bnch, _ = normb_queue.popleft()
            emit_normb(bp, bnch, use_pss=True)
        for p, nch, mt in av_queue:
            emit_av(p, nch, mt)
        lp, lnch = last
        nsl1 = slice(lnch * 512, (lnch + 1) * 512)
        proj_ps = {}

        def proj_head(ot):
            osl = slice(ot * 128, (ot + 1) * 128)
            ps = ps_pool.tile([128, 512], F32, tag="ps", name=f"psyh_{ot}")
            for kt in range(KT - 1):
                nc.tensor.matmul(
                    ps,
                    lhsT=wp[kt][:, osl],
                    rhs=attnT[kt][:, nsl1],
                    start=(kt == 0),
                    stop=False,
                )
            proj_ps[ot] = ps

        def proj_tail(ot):
            osl = slice(ot * 128, (ot + 1) * 128)
            ps = proj_ps.pop(ot)
            nc.tensor.matmul(
                ps,
                lhsT=wp[KT - 1][:, osl],
                rhs=attnT[KT - 1][:, nsl1],
                start=False,
                stop=True,
            )
            y = y_pool.tile([128, 512], F32, tag="y", name=f"yt_{ot}")
            nc.scalar.activation(y, ps, AF.Identity, bias=b_sb[:, ot : ot + 1])
            nc.sync.dma_start(out=out_d.ap()[osl, nsl1], in_=y)

        proj_head(0)
        proj_head(1)
        while normb_queue:
            bp, bnch, _ = normb_queue.popleft()
            emit_normb(bp, bnch, use_pss=True)
        for ot in range(KT):
            proj_tail(ot)
            if ot + 2 < KT:
                proj_head(ot + 2)
        import os
        if os.environ.get("KERNEL_SCHED_DEBUG"):
            print(
                f"[sched] gate starvations: {_stats['starved']} "
                f"({_stats['starve_ns']:.0f} ns), pe={clock['pe']:.0f} "
                f"sc={clock['sc']:.0f}"
            )
            for row in _stats.get("log", []):
                print("[starve]", row)


def _get_nc():
    if "nc" not in _CACHE:
        _CACHE["nc"] = build()
    return _CACHE["nc"]


def kernel(x, w_qkv, w_proj, b_proj, _trace=False):
    import ml_dtypes

    bf16 = ml_dtypes.bfloat16
    x = np.asarray(x, dtype=np.float32)
    w_qkv = np.asarray(w_qkv, dtype=np.float32)
    w_proj = np.asarray(w_proj, dtype=np.float32)
    b_proj = np.asarray(b_proj, dtype=np.float32)

    wq = w_qkv[0:C] * np.float32(SCALE)
    wk = w_qkv[C : 2 * C]
    wv = w_qkv[2 * C : 3 * C]
    w_qkT = np.concatenate([wq, wk], axis=0).T  # [C, 2C]
    # block layout: [ot, p, kt*128+m] = w_qkT[kt*128+p, ot*128+m] so each
    # QKV output tile's weights are one contiguous DMA
    w_qkB = np.ascontiguousarray(
        w_qkT.reshape(KT, 128, 12, 128).transpose(2, 1, 0, 3).reshape(12, 128, C)
    ).astype(bf16)
    w_vT = np.ascontiguousarray(wv.T).astype(bf16)
    w_pT = np.ascontiguousarray(w_proj.T).astype(bf16)
    b_p = np.ascontiguousarray(b_proj.reshape(C, 1))

    in_maps = []
    for i in range(B):
        in_maps.append(
            {
                "xT": np.ascontiguousarray(x[i].T).astype(bf16),  # [C, N]
                "w_qkT": w_qkB,
                "w_vT": w_vT,
                "w_pT": w_pT,
                "b_p": b_p,
            }
        )

    nc = _get_nc()
    res = run_bass_kernel_spmd(nc, in_maps, core_ids=list(range(B)), trace=_trace)
    _CACHE["last_result"] = res

    out = np.empty((B, N, C), dtype=np.float32)
    for i in range(B):
        out[i] = res.results[i]["out"].T
    return out


# revision 44
# speedup vs baseline: 1.0294x; 1.0294x over previous
"""Multi-head attention (B=8, N=1024, C=768, H=12) on 8 TRN2 NeuronCores.

Data-parallel: one batch element per core. Feature-major on chip (no
transposes):

  qkT  = [w_q * scale ; w_k] @ x^T          -> [1536, 1024]  (feature-major)
  V    = x @ w_v^T                          -> [1024, 768]   (token-major)
  S^T  = K_h @ Q_h^T                        -> [128m, 512n] per (pair, m-tile)
  P^T  = exp(S^T)            (no max-sub: scores ~ N(0,1), fp32-safe)
  [out_h ; Z] = [V_h | 1]^T @ P^T           -> [65, 512]  (Z = softmax denom)
  attnT[h] = out_h * recip(Z bcast via K=2 matmul, both heads at once)
  y^T  = w_proj @ attnT + b                 -> [768, 1024]

All inputs are pre-cast to bf16 on the host (fp32 PSUM accumulation on
chip), so DMAs feed SBUF directly with no on-chip cast. The emission
order interleaves score matmuls (which feed ScalarE's exp stream) with
QKV/V/AV/proj "filler" matmuls so the PE never idles: TRN2's tensor
engine downclocks after any gap and takes ~3us to re-ramp, so density
is worth more than locality. A dummy-matmul warmup burst runs during
the initial DMA wait to pre-ramp the PE clock.
"""

import sys

if "/opt/trn_rl_repo" not in sys.path:
    sys.path.insert(0, "/opt/trn_rl_repo")

import numpy as np

import concourse.bass as bass  # noqa: F401
import concourse.mybir as mybir
import concourse.tile as tile
from concourse import bacc
from concourse.bass_utils import run_bass_kernel_spmd

F32 = mybir.dt.float32
F32R = mybir.dt.float32r
BF16 = mybir.dt.bfloat16
AF = mybir.ActivationFunctionType

B, N, C = 8, 1024, 768
H, D = 12, 64
SCALE = D ** -0.5
KT = C // 128            # 6 contraction tiles
NT = N // 128            # 8 token (m) tiles
NCH = N // 512           # 2 free-dim chunks of 512
NPAIR = H // 2           # 6 head pairs

_CACHE = {}


def build():
    nc = bacc.Bacc("TRN2", target_bir_lowering=False, debug=False, num_devices=8)

    xT_d = nc.dram_tensor("xT", [C, N], BF16, kind="ExternalInput")
    wqk_d = nc.dram_tensor("w_qkT", [12, 128, C], BF16, kind="ExternalInput")
    wv_d = nc.dram_tensor("w_vT", [C, C], BF16, kind="ExternalInput")
    wp_d = nc.dram_tensor("w_pT", [C, C], BF16, kind="ExternalInput")
    b_d = nc.dram_tensor("b_p", [C, 1], F32, kind="ExternalInput")
    out_d = nc.dram_tensor("out", [C, N], F32, kind="ExternalOutput")

    with tile.TileContext(nc) as tc:
        _body(nc, tc, xT_d, wqk_d, wv_d, wp_d, b_d, out_d)
    nc.compile()
    return nc


def _body(nc, tc, xT_d, wqk_d, wv_d, wp_d, b_d, out_d):
    from collections import deque
    from contextlib import ExitStack

    with ExitStack() as ctx:
        ctx.enter_context(
            nc.allow_low_precision(reason="bf16 matmul operands; accum stays f32")
        )
        const = ctx.enter_context(tc.tile_pool(name="const", bufs=1))
        x_pool = ctx.enter_context(tc.tile_pool(name="x", bufs=1))
        w_pool = ctx.enter_context(tc.tile_pool(name="w", bufs=1))
        qk_pool = ctx.enter_context(tc.tile_pool(name="qk", bufs=1))
        v_pool = ctx.enter_context(tc.tile_pool(name="v", bufs=1))
        attn_pool = ctx.enter_context(tc.tile_pool(name="attn", bufs=1))
        pt_pool = ctx.enter_context(tc.tile_pool(name="pt", bufs=36))
        zs_pool = ctx.enter_context(tc.tile_pool(name="zs", bufs=2))
        y_pool = ctx.enter_context(tc.tile_pool(name="y", bufs=3))
        ps_pool = ctx.enter_context(tc.tile_pool(name="ps", bufs=2, space="PSUM"))
        pss_pool = ctx.enter_context(tc.tile_pool(name="pss", bufs=2, space="PSUM"))
        pav_pool = ctx.enter_context(tc.tile_pool(name="pav", bufs=2, space="PSUM"))

        # ---------------- constants ----------------
        warm = const.tile([128, 512], BF16)
        nc.vector.memset(warm, 0.25)
        b_sb = const.tile([128, KT], F32)
        # ---------------- persistent SBUF tensors ----------------
        xT = [x_pool.tile([128, N], BF16, tag=f"x{i}", name=f"x{i}") for i in range(KT)]
        wqk = [w_pool.tile([128, KT * 128], BF16, tag=f"wqk{i}", name=f"wqk{i}") for i in range(12)]
        wv = [w_pool.tile([128, C], BF16, tag=f"wv{i}", name=f"wv{i}") for i in range(KT)]
        wp = [w_pool.tile([128, C], BF16, tag=f"wp{i}", name=f"wp{i}") for i in range(KT)]
        qkT = [qk_pool.tile([128, N], BF16, tag=f"qkT{i}", name=f"qkT{i}") for i in range(12)]
        v_sb = [v_pool.tile([128, H, 65], BF16, tag=f"v{i}", name=f"v{i}") for i in range(NT)]
        attnT = [attn_pool.tile([128, N], BF16, tag=f"at{i}", name=f"at{i}") for i in range(KT)]

        # ---------------- input DMAs (2 queues, consumption order) ----------------
        # sync queue: xT (needed first), then wv, b; scalar queue: wqk blocks
        # in the order QKV tiles consume them, then wp (needed last).
        for kt in range(KT):
            ksl = slice(kt * 128, (kt + 1) * 128)
            nc.sync.dma_start(out=xT[kt], in_=xT_d.ap()[ksl, :])
        for ot in (0, 6, 1, 7, 2, 8, 3, 9, 4, 10, 5, 11):
            nc.scalar.dma_start(
                out=wqk[ot].rearrange("p (k m) -> p k m", m=128),
                in_=wqk_d.ap()[ot],
            )
        for kt in range(KT):
            ksl = slice(kt * 128, (kt + 1) * 128)
            nc.sync.dma_start(out=wv[kt], in_=wv_d.ap()[ksl, :])
        for ot in range(KT):
            nc.sync.dma_start(
                out=b_sb[:, ot : ot + 1], in_=b_d.ap()[ot * 128 : (ot + 1) * 128, :]
            )
        for kt in range(KT):
            ksl = slice(kt * 128, (kt + 1) * 128)
            nc.scalar.dma_start(out=wp[kt], in_=wp_d.ap()[ksl, :])
        # V ones-columns (row 64 of each head slot)
        for mt in range(NT):
            nc.gpsimd.memset(v_sb[mt][:, :, 64:65], 1.0)

        # ---------------- more constants (gpsimd, after DMA issues) ----------------
        # Z-broadcast stationary: row 0 -> out partitions 0..63, row 64 ->
        # out partitions 64..127 (partition bases must be quadrant-aligned).
        zst_raw = const.tile([128, 128], F32)
        nc.gpsimd.memset(zst_raw, 0.0)
        nc.gpsimd.memset(zst_raw[0:1, 0:64], 1.0)
        nc.gpsimd.memset(zst_raw[64:65, 64:128], 1.0)
        zst = const.tile([128, 128], F32R)
        nc.gpsimd.tensor_copy(zst, zst_raw)
        # Z staging tiles (rows 0 and 64 carry the two heads' denominators;
        # the rest is zeroed once and multiplied by zero weights anyway).
        z2_zero = const.tile([128, 512], F32)
        nc.gpsimd.memset(z2_zero, 0.0)
        z2_bufs = [const.tile([128, 512], F32R, name=f"z2_{i}") for i in range(2)]
        for zb_ in z2_bufs:
            nc.gpsimd.tensor_copy(zb_, z2_zero)

        # ---------------- PE warmup (overlaps DMA wait) ----------------
        for i in range(12):
            pw = pss_pool.tile([128, 1024], F32, tag="pss", name=f"warm{i}")
            nc.tensor.matmul(
                pw[:, 0:512], lhsT=warm[:, 0:128], rhs=warm, start=True, stop=True
            )

        # ---------------- work units ----------------
        def qku(ot, nch):
            """One QKV output tile chunk: 6 matmuls + copy to qkT."""
            nsl = slice(nch * 512, (nch + 1) * 512)
            ps = ps_pool.tile([128, 512], F32, tag="ps", name=f"psqk_{ot}_{nch}")
            for kt in range(KT):
                nc.tensor.matmul(
                    ps,
                    lhsT=wqk[ot][:, kt * 128 : (kt + 1) * 128],
                    rhs=xT[kt][:, nsl],
                    start=(kt == 0),
                    stop=(kt == KT - 1),
                )
            nc.vector.tensor_copy(qkT[ot][:, nsl], ps)

        def v_unit(mt):
            """V for one token tile: x-tile stationary, wv moving (512+256)."""
            msl = slice(mt * 128, (mt + 1) * 128)
            for o0, ow in ((0, 512), (512, 256)):
                nh = ow // 64
                ps = ps_pool.tile([128, 512], F32, tag="ps", name=f"psv_{mt}_{o0}")
                for kt in range(KT):
                    nc.tensor.matmul(
                        ps[:, :ow],
                        lhsT=xT[kt][:, msl],
                        rhs=wv[kt][:, o0 : o0 + ow],
                        start=(kt == 0),
                        stop=(kt == KT - 1),
                    )
                dst = v_sb[mt][:, o0 // 64 : o0 // 64 + nh, 0:64]
                vsrc = ps[:, :ow].rearrange("p (h e) -> p h e", e=64)
                nc.vector.tensor_copy(dst, vsrc)

        pt_tiles = {}

        def s_unit(p, nch, g):
            """Scores + exp for one (pair, n-chunk, m-tile-pair): 4 matmuls +
            2 exps of [128, 1024] (bigger exps amortize ScalarE overhead)."""
            q_t = qkT[p]
            k_t = qkT[6 + p]
            nsl = slice(nch * 512, (nch + 1) * 512)
            for e in range(2):
                esl = slice(e * 64, e * 64 + 64)
                pss = pss_pool.tile(
                    [128, 1024], F32, tag="pss", name=f"pss_{p}_{nch}_{g}_{e}"
                )
                for j in range(2):
                    mt = 2 * g + j
                    msl = slice(mt * 128, (mt + 1) * 128)
                    nc.tensor.matmul(
                        pss[:, j * 512 : (j + 1) * 512],
                        lhsT=k_t[esl, msl],
                        rhs=q_t[esl, nsl],
                        start=True,
                        stop=True,
                    )
                pt = pt_pool.tile(
                    [128, 1024], BF16, tag="pt", name=f"pt_{p}_{nch}_{g}_{e}"
                )
                nc.scalar.activation(pt, pss, AF.Exp)
                pt_tiles[(p, nch, g, e)] = pt

        pav_cur = {}

        def av_unit(p, nch, mt):
            """AV accumulation for one m-tile of a block (2 matmuls)."""
            if mt == 0:
                pav_cur[(p, nch)] = [
                    pav_pool.tile([65, 512], F32, tag="pav", name=f"pav_{p}_{nch}_{e}")
                    for e in range(2)
                ]
            pav = pav_cur[(p, nch)]
            g, j = divmod(mt, 2)
            jsl = slice(j * 512, (j + 1) * 512)
            for e in range(2):
                h = 2 * p + e
                pt = pt_tiles[(p, nch, g, e)]
                if j == 1 and e == 1:
                    del pt_tiles[(p, nch, g, 0)], pt_tiles[(p, nch, g, 1)]
                nc.tensor.matmul(
                    pav[e],
                    lhsT=v_sb[mt][:, h, :],
                    rhs=pt[:, jsl],
                    start=(mt == 0),
                    stop=(mt == NT - 1),
                )

        norm_count = [0]
        tail_mode = [False]
        z2_of = {}

        def norm_a(p, nch):
            """First half of softmax normalization: stage Z rows (no PE)."""
            pav = pav_cur[(p, nch)]
            z2 = z2_bufs[norm_count[0] % 2]
            norm_count[0] += 1
            z2_of[(p, nch)] = z2
            if tail_mode[0]:
                nc.scalar.copy(out=z2[0:1, :], in_=pav[0][64:65, :])
                nc.scalar.copy(out=z2[64:65, :], in_=pav[1][64:65, :])
            else:
                nc.vector.tensor_copy(z2[0:1, :], pav[0][64:65, :])
                nc.vector.tensor_copy(z2[64:65, :], pav[1][64:65, :])

        def norm_b(p, nch, use_pss=False):
            """Second half: Z broadcast matmul + reciprocal + muls."""
            nsl = slice(nch * 512, (nch + 1) * 512)
            pav = pav_cur.pop((p, nch))
            z2 = z2_of.pop((p, nch))
            if use_pss:
                zbt = pss_pool.tile([128, 1024], F32, tag="pss", name=f"zb_{p}_{nch}")
                zb = zbt[:, 0:512]
            else:
                zb = ps_pool.tile([128, 512], F32, tag="ps", name=f"zb_{p}_{nch}")
            nc.tensor.matmul(zb, lhsT=zst, rhs=z2, start=True, stop=True)
            zr = zs_pool.tile([128, 512], F32, tag="zr", name=f"zr_{p}_{nch}")
            nc.vector.reciprocal_approx_fast(out=zr, in_=zb)
            for e in range(2):
                dst = attnT[p][e * 64 : e * 64 + 64, nsl]
                nc.vector.tensor_mul(dst, zr[e * 64 : e * 64 + 64, :], pav[e][0:64, :])

        def proj_unit(ot, nch):
            """Output projection tile: 6 matmuls + bias add (vector) + DMA out."""
            osl = slice(ot * 128, (ot + 1) * 128)
            nsl = slice(nch * 512, (nch + 1) * 512)
            ps = ps_pool.tile([128, 512], F32, tag="ps", name=f"psy_{ot}_{nch}")
            for kt in range(KT):
                nc.tensor.matmul(
                    ps,
                    lhsT=wp[kt][:, osl],
                    rhs=attnT[kt][:, nsl],
                    start=(kt == 0),
                    stop=(kt == KT - 1),
                )
            y = y_pool.tile([128, 512], F32, tag="y", name=f"y_{ot}_{nch}")
            if tail_mode[0]:
                nc.scalar.activation(y, ps, AF.Identity, bias=b_sb[:, ot : ot + 1])
            else:
                nc.vector.tensor_scalar_add(y, ps, b_sb[:, ot : ot + 1])
            nc.sync.dma_start(out=out_d.ap()[osl, nsl], in_=y)

        # ---------------- schedule ----------------
        # Emission order is engine program order. A coarse clock model
        # (pe_t = estimated PE stream position, sc_t = estimated ScalarE exp
        # completion) rations filler so the PE reaches each scores unit just
        # as its PSUM slot is freed by the exp stream -- never stalling
        # (which would also drop the PE out of max p-state).
        MM512, MM256, EXPNS, MARGIN = 215.0, 110.0, 1180.0, -500.0
        _stats = {"starved": 0, "starve_ns": 0.0}
        clock = {"pe": 2500.0, "sc": 0.0}
        pss_free = [0.0] * 12  # warmup allocations, free immediately
        exp_end = {}
        for _ in range(12):
            clock["pe"] += MM512

        blocks = [(p, nch) for nch in range(NCH) for p in range(NPAIR)]
        qku_done = set()

        def emit_qku(ot, nch):
            qku(ot, nch)
            qku_done.add((ot, nch))
            clock["pe"] += KT * MM512

        qku_queue = deque(
            (ot, nch)
            for p in range(1, NPAIR)
            for ot in (p, 6 + p)
            for nch in range(NCH)
        )
        v_queue = deque(range(NT))
        av_queue = deque()
        proj_queue = deque()
        v_emitted = 0
        norms_done = [0] * NCH
        normb_queue = deque()  # (p, nch, pe_when_staged)

        def emit_av(p, nch, mt):
            av_unit(p, nch, mt)
            clock["pe"] += 2 * MM512
            if mt == NT - 1:
                norm_a(p, nch)
                normb_queue.append((p, nch, clock["pe"]))

        def emit_normb(p, nch, use_pss=False):
            norm_b(p, nch, use_pss)
            clock["pe"] += MM512
            norms_done[nch] += 1
            if nch == 0 and norms_done[0] == NPAIR:
                proj_queue.extend((ot, 0) for ot in range(KT))

        def emit_proj(ot, nch):
            proj_unit(ot, nch)
            clock["pe"] += KT * MM512

        dummy_n = [0]

        def emit_dummy():
            # throwaway matmul: keeps the PE at max p-state through a gap
            # that has no real work (a stall would cost ~3us of half-clock)
            ps = ps_pool.tile([128, 512], F32, tag="ps", name=f"dum{dummy_n[0]}")
            dummy_n[0] += 1
            nc.tensor.matmul(
                ps, lhsT=warm[:, 0:128], rhs=warm, start=True, stop=True
            )
            clock["pe"] += MM512

        def emit_v():
            nonlocal v_emitted
            v_unit(v_queue.popleft())
            v_emitted += 1
            clock["pe"] += KT * (MM512 + MM256)

        def emit_filler(upcoming_pair):
            if normb_queue and (
                clock["pe"] >= normb_queue[0][2] + 1600 or len(normb_queue) >= 2
            ):
                p, nch, _ = normb_queue.popleft()
                emit_normb(p, nch)
                return True
            if qku_queue and qku_queue[0][0] in (upcoming_pair, 6 + upcoming_pair):
                emit_qku(*qku_queue.popleft())
                return True
            if v_queue:
                emit_v()
                return True
            if av_queue and v_emitted == NT:
                p, nch, mt = av_queue[0]
                g = mt // 2
                if mt == 0 and normb_queue:
                    # new block needs both pav slots: flush pending norm-b
                    bp, bnch, _ = normb_queue.popleft()
                    emit_normb(bp, bnch)
                    return True
                if exp_end[(p, nch, g, 1)] <= clock["pe"]:
                    av_queue.popleft()
                    emit_av(p, nch, mt)
                    return True
            if proj_queue:
                emit_proj(*proj_queue.popleft())
                return True
            if qku_queue:
                emit_qku(*qku_queue.popleft())
                return True
            return False

        def s_half(p, nch, g, e):
            """Scores matmuls + exp for one head of a score group, with
            filler emitted until the needed pss slot is modeled free."""
            need = pss_free[len(pss_free) - 2] - MARGIN
            while clock["pe"] < need:
                if not emit_filler(p):
                    _stats["starved"] += 1
                    _stats["starve_ns"] += need - clock["pe"]
                    _stats.setdefault("log", []).append(
                        (p, nch, g, e, round(need - clock["pe"]),
                         len(qku_queue), len(v_queue), len(av_queue),
                         len(proj_queue), len(normb_queue),
                         av_queue[0] if av_queue else None,
                         round(clock["pe"]))
                    )
                    break
            q_t = qkT[p]
            k_t = qkT[6 + p]
            nsl = slice(nch * 512, (nch + 1) * 512)
            esl = slice(e * 64, e * 64 + 64)
            pss = pss_pool.tile(
                [128, 1024], F32, tag="pss", name=f"pss_{p}_{nch}_{g}_{e}"
            )
            for j in range(2):
                mt = 2 * g + j
                msl = slice(mt * 128, (mt + 1) * 128)
                nc.tensor.matmul(
                    pss[:, j * 512 : (j + 1) * 512],
                    lhsT=k_t[esl, msl],
                    rhs=q_t[esl, nsl],
                    start=True,
                    stop=True,
                )
            clock["pe"] += 2 * MM512
            pt = pt_pool.tile(
                [128, 1024], BF16, tag="pt", name=f"pt_{p}_{nch}_{g}_{e}"
            )
            nc.scalar.activation(pt, pss, AF.Exp)
            end = max(clock["sc"], clock["pe"]) + EXPNS
            clock["sc"] = end
            pss_free.append(end)
            exp_end[(p, nch, g, e)] = end
            pt_tiles[(p, nch, g, e)] = pt

        # pair-0 Q/K first so the exp stream starts as early as possible
        for ot in (0, 6):
            for nch in range(NCH):
                emit_qku(ot, nch)

        for bi, (p, nch) in enumerate(blocks):
            nxt = blocks[bi + 1][0] if bi + 1 < len(blocks) else NPAIR - 1
            for ot in (p, 6 + p):
                for c in range(NCH):
                    if (ot, c) not in qku_done:
                        qku_queue.remove((ot, c))
                        emit_qku(ot, c)
            for g in range(NT // 2):
                for e in range(2):
                    s_half(p, nch, g, e)
            if bi >= 1:
                bp, bnch = blocks[bi - 1]
                av_queue.extend((bp, bnch, mt) for mt in range(NT))

        # ---------------- tail ----------------
        for bp, bnch in blocks[-1:]:
            av_queue.extend((bp, bnch, mt) for mt in range(NT))
        # drain AV (except the final block); between a block's norm-a (z2
        # copies) and norm-b (zb matmul) give the PE proj work so it never
        # waits on the copy/recip chain
        last = blocks[-1]
        while av_queue:
            p, nch, mt = av_queue[0]
            if (p, nch) == last:
                break
            av_queue.popleft()
            emit_av(p, nch, mt)
            if mt == NT - 1:
                if proj_queue:
                    emit_proj(*proj_queue.popleft())
                if normb_queue:
                    bp, bnch, _ = normb_queue.popleft()
                    emit_normb(bp, bnch, use_pss=True)
                if proj_queue:
                    emit_proj(*proj_queue.popleft())
        while proj_queue:
            emit_proj(*proj_queue.popleft())
        # final block: AV, then proj(nch=1) split into head (kt 0..4, does
        # not need the final norm) and tail (kt=5 + bias + DMA out) so the
        # PE has work while the last norm's vector chain completes. Only now
        # is ScalarE's exp backlog drained enough to take the copies/bias.
        tail_mode[0] = True
        while len(normb_queue) > 0 and av_queue:
            bp, bnch, _ = normb_queue.popleft()
            emit_normb(bp, bnch, use_pss=True)
        for p, nch, mt in av_queue:
            emit_av(p, nch, mt)
        lp, lnch = last
        nsl1 = slice(lnch * 512, (lnch + 1) * 512)
        proj_ps = {}

        def proj_head(ot):
            osl = slice(ot * 128, (ot + 1) * 128)
            ps = ps_pool.tile([128, 512], F32, tag="ps", name=f"psyh_{ot}")
            for kt in range(KT - 1):
                nc.tensor.matmul(
                    ps,
                    lhsT=wp[kt][:, osl],
                    rhs=attnT[kt][:, nsl1],
                    start=(kt == 0),
                    stop=False,
                )
            proj_ps[ot] = ps

        def proj_tail(ot):
            osl = slice(ot * 128, (ot + 1) * 128)
            ps = proj_ps.pop(ot)
            nc.tensor.matmul(
                ps,
                lhsT=wp[KT - 1][:, osl],
                rhs=attnT[KT - 1][:, nsl1],
                start=False,
                stop=True,
            )
            y = y_pool.tile([128, 512], F32, tag="y", name=f"yt_{ot}")
            nc.scalar.activation(y, ps, AF.Identity, bias=b_sb[:, ot : ot + 1])
            nc.sync.dma_start(out=out_d.ap()[osl, nsl1], in_=y)

        proj_head(0)
        proj_head(1)
        while normb_queue:
            bp, bnch, _ = normb_queue.popleft()
            emit_normb(bp, bnch, use_pss=True)
        for ot in range(KT):
            proj_tail(ot)
            if ot + 2 < KT:
                proj_head(ot + 2)
        import os
        if os.environ.get("KERNEL_SCHED_DEBUG"):
            print(
                f"[sched] gate starvations: {_stats['starved']} "
                f"({_stats['starve_ns']:.0f} ns), pe={clock['pe']:.0f} "
                f"sc={clock['sc']:.0f}"
            )
            for row in _stats.get("log", []):
                print("[starve]", row)


def _get_nc():
    if "nc" not in _CACHE:
        _CACHE["nc"] = build()
    return _CACHE["nc"]


def kernel(x, w_qkv, w_proj, b_proj, _trace=False):
    import ml_dtypes

    bf16 = ml_dtypes.bfloat16
    x = np.asarray(x, dtype=np.float32)
    w_qkv = np.asarray(w_qkv, dtype=np.float32)
    w_proj = np.asarray(w_proj, dtype=np.float32)
    b_proj = np.asarray(b_proj, dtype=np.float32)

    wq = w_qkv[0:C] * np.float32(SCALE)
    wk = w_qkv[C : 2 * C]
    wv = w_qkv[2 * C : 3 * C]
    w_qkT = np.concatenate([wq, wk], axis=0).T  # [C, 2C]
    # block layout: [ot, p, kt*128+m] = w_qkT[kt*128+p, ot*128+m] so each
    # QKV output tile's weights are one contiguous DMA
    w_qkB = np.ascontiguousarray(
        w_qkT.reshape(KT, 128, 12, 128).transpose(2, 1, 0, 3).reshape(12, 128, C)
    ).astype(bf16)
    w_vT = np.ascontiguousarray(wv.T).astype(bf16)
    w_pT = np.ascontiguousarray(w_proj.T).astype(bf16)
    b_p = np.ascontiguousarray(b_proj.reshape(C, 1))

    in_maps = []
    for i in range(B):
        in_maps.append(
            {
                "xT": np.ascontiguousarray(x[i].T).astype(bf16),  # [C, N]
                "w_qkT": w_qkB,
                "w_vT": w_vT,
                "w_pT": w_pT,
                "b_p": b_p,
            }
        )

    nc = _get_nc()
    res = run_bass_kernel_spmd(nc, in_maps, core_ids=list(range(B)), trace=_trace)
    _CACHE["last_result"] = res

    out = np.empty((B, N, C), dtype=np.float32)
    for i in range(B):
        out[i] = res.results[i]["out"].T
    return out
